# revision 8
# baseline (speedup 1.0000x reference)
"""Trainium2 Bass kernel for nn_NeuralClustering (n==1 path, K==1).

Data-parallel over 8 NeuronCores: batch 1024 -> 128 per core; the small
h/g/f MLP weights are replicated. All activations are kept feature-major
([features=partitions, rows=free]) so PE matmuls contract over partitions,
biases are per-partition ACT operands, and PReLU is a single fused ACT
`parametric_relu` on the PSUM->SBUF drain. Matmuls run in float32r.
"""
import numpy as np

import concourse.bass as bass
import concourse.mybir as mybir
import concourse.tile as tile
from concourse.bass_utils import run_bass_kernel_spmd

from wait_split import split_waits

F32 = mybir.dt.float32
F32R = mybir.dt.float32r
AF = mybir.ActivationFunctionType
ALU = mybir.AluOpType

B, N, XD = 1024, 100, 2
HD, GD, HID = 256, 512, 512
NCORES = 8
BL = B // NCORES            # 128 batch items per core
BGRP = 4                    # batch items per h-block
RBLK = BGRP * N             # 400 rows per h-block
NBLK = BL // BGRP           # 32 h-blocks
NG = 3 * BL                 # 384 g rows (h0+hn | h0 | hn)
NF = 2 * BL                 # 256 f rows (cand-major)

H_DIMS = (XD, HID, HID, HID, HID, HD)
G_DIMS = (HD, HID, HID, HID, HID, HID, GD)
F_DIMS = (GD + 2 * HD, HID, HID, HID, HID, HID, 1)


def _declare_weights(nc, prefix, dims, last_bias):
    ws, bs = [], []
    for i, (di, do) in enumerate(zip(dims[:-1], dims[1:])):
        ws.append(nc.declare_dram_parameter(f"{prefix}W{i}", [di, do], F32R, False))
        if i < len(dims) - 2 or last_bias:
            bs.append(nc.declare_dram_parameter(f"{prefix}b{i}", [do], F32, False))
        else:
            bs.append(None)
    return ws, bs


def _load_weights(nc, pool, ws, bs):
    """DMA weights into SBUF as [128, nk*do] lhsT chunk tiles, biases [128, nm]."""
    wts, bts = [], []
    for i, w in enumerate(ws):
        di, do = w.shape
        if di < 128:
            t = pool.tile([di, do], F32R, tag=f"w{w.name}")
            nc.sync.dma_start(t[:], w[:])
        else:
            nk = di // 128
            t = pool.tile([128, nk * do], F32R, tag=f"w{w.name}")
            nc.sync.dma_start(
                t[:].rearrange("p (k c) -> p k c", k=nk),
                w[:].rearrange("(k p) c -> p k c", p=128),
            )
        wts.append(t)
        b = bs[i]
        if b is None:
            bts.append(None)
            continue
        do = b.shape[0]
        if do >= 128:
            nm = do // 128
            bt = pool.tile([128, nm], F32, tag=f"b{b.name}")
            nc.sync.dma_start(bt[:], b[:].rearrange("(m p) -> p m", p=128))
        else:
            bt = pool.tile([do, 1], F32, tag=f"b{b.name}")
            nc.sync.dma_start(bt[:], b[:].rearrange("p -> p 1"))
        bts.append(bt)
    return wts, bts


def _mlp_layer(nc, ps_pool, act_pool, wt, bt, act_in, nrows, di, do,
               out_dtype=F32R, func=AF.Prelu, out_tile=None, tag=None):
    """One feature-major linear(+bias) layer with fused activation.

    act_in: SBUF tile [128, nk*nrows] (k-chunk c at cols [c*nrows,(c+1)*nrows)).
    Returns SBUF tile [128, nm*nrows] in out_dtype.
    """
    nk = max(1, di // 128)
    nm = (do + 127) // 128
    mm = min(do, 128)
    if out_tile is None:
        out_tile = act_pool.tile([128, nm * nrows], out_dtype, tag=tag)
    psums = []
    for _m in range(nm):
        pt = ps_pool.tile([mm, nrows] if mm < 128 else [128, nrows], F32,
                          tag="ps", name=f"ps_{_m}")
        psums.append(pt)
    for k in range(nk):
        if di < 128:  # single short-K chunk (h layer 0: K=2)
            rhs = act_in[:]
        else:
            rhs = act_in[:, k * nrows:(k + 1) * nrows]
        for m in range(nm):
            lhsT = wt[:, k * do + m * mm: k * do + (m + 1) * mm]
            nc.tensor.matmul(psums[m][:], lhsT, rhs,
                             start=(k == 0), stop=(k == nk - 1))
    for m in range(nm):
        dst = out_tile[:, m * nrows:(m + 1) * nrows] if mm == 128 else out_tile[:]
        bias = bt[:, m:m + 1] if bt is not None else 0.0
        nc.scalar.activation(dst, psums[m][:], func, bias=bias, scale=1.0,
                             alpha=0.25)
    return out_tile


def build_program(loop_reps=None):
    nc = bass.Bass()
    data = nc.declare_dram_parameter("data", [BL, N, XD], F32R, False)
    out = nc.declare_dram_parameter("out", [2, BL], F32, True)

    hW, hB = _declare_weights(nc, "h", H_DIMS, True)
    gW, gB = _declare_weights(nc, "g", G_DIMS, True)
    fW, fB = _declare_weights(nc, "f", F_DIMS, False)

    from contextlib import ExitStack
    with tile.TileContext(nc) as tc, ExitStack() as ctx:
        dump_sem = nc.alloc_semaphore("wsp_dump")
        wpool = ctx.enter_context(tc.tile_pool(name="weights", bufs=1))
        apool = ctx.enter_context(tc.tile_pool(name="acts", bufs=2))
        spool = ctx.enter_context(tc.tile_pool(name="stats", bufs=1))
        xpool = ctx.enter_context(tc.tile_pool(name="xt", bufs=3))
        hspool = ctx.enter_context(tc.tile_pool(name="hs", bufs=2))
        pspool = ctx.enter_context(tc.tile_pool(name="psum", bufs=8, space="PSUM"))

        hWt, hBt = _load_weights(nc, wpool, hW, hB)
        gWt, gBt = _load_weights(nc, wpool, gW, gB)
        fWt, fBt = _load_weights(nc, wpool, fW, fB)

        if loop_reps is not None:
            loop_cm = tc.For_i(0, loop_reps, 1)
            loop_cm.__enter__()

        # Accumulators across h blocks (feature-major [128, 2*BL]; HD=256 -> 2 chunks)
        nmh = HD // 128
        q_sum = spool.tile([128, nmh * BL], F32, tag="q_sum")
        h0_all = spool.tile([128, nmh * BL], F32, tag="h0")
        hn_all = spool.tile([128, nmh * BL], F32, tag="hn")

        data_fbn = data[:].rearrange("b n f -> f b n")

        for blk in range(NBLK):
            xt = xpool.tile([XD, RBLK], F32R, tag="xt")
            nc.sync.dma_start(
                xt[:].rearrange("f (b n) -> f b n", b=BGRP),
                data_fbn[:, blk * BGRP:(blk + 1) * BGRP, :],
            )
            a = _mlp_layer(nc, pspool, apool, hWt[0], hBt[0], xt, RBLK,
                           XD, HID, tag="hact")
            for l in (1, 2, 3):
                a = _mlp_layer(nc, pspool, apool, hWt[l], hBt[l], a, RBLK,
                               HID, HID, tag="hact")
            hs = _mlp_layer(nc, pspool, hspool, hWt[4], hBt[4], a, RBLK,
                            HID, HD, out_dtype=F32, func=AF.Identity, tag="hs")
            # per-block reduction: sum over n, extract n=0 / n=1
            for m in range(nmh):
                col = m * BL + blk * BGRP
                nc.vector.tensor_reduce(
                    q_sum[:, col:col + BGRP],
                    hs[:, m * RBLK:(m + 1) * RBLK].rearrange(
                        "p (b n) -> p b n", b=BGRP),
                    mybir.AxisListType.X, ALU.add,
                )
                src = hs[:].rearrange("p (m b n) -> p m b n", m=nmh, b=BGRP)
                nc.vector.tensor_copy(h0_all[:, col:col + BGRP], src[:, m, :, 0])
                nc.vector.tensor_copy(hn_all[:, col:col + BGRP], src[:, m, :, 1])

        # Q = sum_{n>=2} = q_sum - h0 - hn   (keep f32)
        q = spool.tile([128, nmh * BL], F32, tag="q")
        nc.vector.tensor_tensor(q[:], q_sum[:], h0_all[:], ALU.subtract)
        nc.vector.tensor_tensor(q[:], q[:], hn_all[:], ALU.subtract)

        # g input [128, nmh * 384]: chunks (h0+hn | h0 | hn) per feature-half
        g_in = spool.tile([128, nmh * NG], F32R, tag="g_in")
        for m in range(nmh):
            base = m * NG
            mh = slice(m * BL, (m + 1) * BL)
            nc.vector.tensor_tensor(g_in[:, base:base + BL],
                                    h0_all[:, mh], hn_all[:, mh], ALU.add)
            nc.vector.tensor_copy(g_in[:, base + BL:base + 2 * BL], h0_all[:, mh])
            nc.vector.tensor_copy(g_in[:, base + 2 * BL:base + 3 * BL], hn_all[:, mh])

        a = g_in
        dims = G_DIMS
        for l in range(6):
            last = l == 5
            a = _mlp_layer(nc, pspool, apool, gWt[l], gBt[l], a, NG,
                           dims[l], dims[l + 1],
                           out_dtype=F32 if last else F32R,
                           func=AF.Identity if last else AF.Prelu,
                           tag="gact")

        g_out = a  # [128, 4*NG] f32

        # f input [128, 8 * 256] f32r: chunks 0-3 Gk, 4-5 Q, 6-7 hn
        f_in = spool.tile([128, 8 * NF], F32R, tag="f_in")
        nmg = GD // 128
        for kc in range(nmg):
            gbase = kc * NG
            fbase = kc * NF
            nc.vector.tensor_copy(f_in[:, fbase:fbase + BL],
                                  g_out[:, gbase:gbase + BL])
            nc.vector.tensor_tensor(f_in[:, fbase + BL:fbase + 2 * BL],
                                    g_out[:, gbase + BL:gbase + 2 * BL],
                                    g_out[:, gbase + 2 * BL:gbase + 3 * BL],
                                    ALU.add)
        for m in range(nmh):
            fbase = (nmg + m) * NF
            mh = slice(m * BL, (m + 1) * BL)
            nc.vector.tensor_copy(f_in[:, fbase:fbase + BL], q[:, mh])
            nc.vector.tensor_copy(f_in[:, fbase + BL:fbase + 2 * BL], q[:, mh])
            fbase = (nmg + nmh + m) * NF
            nc.vector.tensor_copy(f_in[:, fbase:fbase + BL], hn_all[:, mh])
            nc.vector.tensor_copy(f_in[:, fbase + BL:fbase + 2 * BL], hn_all[:, mh])

        a = f_in
        dims = F_DIMS
        for l in range(5):
            a = _mlp_layer(nc, pspool, apool, fWt[l], fBt[l], a, NF,
                           dims[l], dims[l + 1], tag="fact")
        # final layer 512 -> 1, bias-free: logits psum [1, 256]
        lp = pspool.tile([1, NF], F32, tag="ps")
        for k in range(4):
            nc.tensor.matmul(lp[:], fWt[5][:, k:k + 1],
                             a[:, k * NF:(k + 1) * NF],
                             start=(k == 0), stop=(k == 3))

        # log_softmax over the 2 candidates (cols [0:BL) vs [BL:2BL))
        sm = spool.tile([1, 8 * BL], F32, tag="sm")
        l0 = sm[:, 0 * BL:1 * BL]; l1 = sm[:, 1 * BL:2 * BL]
        mx = sm[:, 2 * BL:3 * BL]; d0 = sm[:, 3 * BL:4 * BL]
        d1 = sm[:, 4 * BL:5 * BL]; e0 = sm[:, 5 * BL:6 * BL]
        e1 = sm[:, 6 * BL:7 * BL]; ls = sm[:, 7 * BL:8 * BL]
        nc.vector.tensor_copy(l0, lp[:, 0:BL])
        nc.vector.tensor_copy(l1, lp[:, BL:2 * BL])
        nc.vector.tensor_tensor(mx, l0, l1, ALU.max)
        nc.vector.tensor_tensor(d0, l0, mx, ALU.subtract)
        nc.vector.tensor_tensor(d1, l1, mx, ALU.subtract)
        nc.scalar.activation(e0, d0, AF.Exp)
        nc.scalar.activation(e1, d1, AF.Exp)
        nc.vector.tensor_tensor(e0, e0, e1, ALU.add)
        nc.scalar.activation(ls, e0, AF.Ln)
        ob = spool.tile([1, 2 * BL], F32, tag="ob")
        nc.vector.tensor_tensor(ob[:, 0:BL], d0, ls, ALU.subtract)
        nc.vector.tensor_tensor(ob[:, BL:2 * BL], d1, ls, ALU.subtract)
        nc.sync.dma_start(out[:].rearrange("a b -> (a b)").rearrange("(o c) -> o c", o=1), ob[:])

        if loop_reps is not None:
            loop_cm.__exit__(None, None, None)

    split_waits(nc, dump_sem)
    return nc


_nc_cache = {}


def _get_nc(loop_reps=None):
    if loop_reps not in _nc_cache:
        _nc_cache[loop_reps] = build_program(loop_reps)
    return _nc_cache[loop_reps]


def kernel(data, cs, n, h_params, g_params, f_params, loop_reps=None, **_ignored):
    nc = _get_nc(loop_reps)
    data = np.ascontiguousarray(np.asarray(data, dtype=np.float32))

    base = {}
    for prefix, params in (("h", h_params), ("g", g_params), ("f", f_params)):
        ws, bs = params[0], params[1]
        for i, w in enumerate(ws):
            base[f"{prefix}W{i}"] = np.ascontiguousarray(np.asarray(w, np.float32))
        for i, b in enumerate(bs):
            if b is not None:
                base[f"{prefix}b{i}"] = np.ascontiguousarray(np.asarray(b, np.float32))

    import os
    in_maps = [dict(base, data=data[c * BL:(c + 1) * BL]) for c in range(NCORES)]
    trace = bool(int(os.environ.get("KERNEL_TRACE", "0")))
    res = run_bass_kernel_spmd(nc, in_maps, list(range(NCORES)), trace=trace)
    global _last_exec_ns, _last_results
    _last_exec_ns = res.exec_time_ns
    _last_results = res
    outp = np.empty((B, 2), dtype=np.float32)
    for c in range(NCORES):
        outp[c * BL:(c + 1) * BL, :] = res.results[c]["out"].T
    return outp


# revision 9
# speedup vs baseline: 3.7268x; 3.7268x over previous
"""Trainium2 Bass kernel for nn_NeuralClustering (n==1 path, K==1).

Data-parallel over 8 NeuronCores: batch 1024 -> 128 per core; the small
h/g/f MLP weights are replicated. All activations are kept feature-major
([features=partitions, rows=free]) so PE matmuls contract over partitions,
biases are per-partition ACT operands, and PReLU is a single fused ACT
`parametric_relu` on the PSUM->SBUF drain. Matmuls run in float32r.
"""
import numpy as np

import concourse.bass as bass
import concourse.mybir as mybir
import concourse.tile as tile
from concourse.bass_utils import run_bass_kernel_spmd

from wait_split import split_waits

F32 = mybir.dt.float32
F32R = mybir.dt.float32r
BF16 = mybir.dt.bfloat16
AF = mybir.ActivationFunctionType
ALU = mybir.AluOpType

import os
MMDT = BF16 if os.environ.get("KERNEL_MMDT", "f32r") == "bf16" else F32R

B, N, XD = 1024, 100, 2
HD, GD, HID = 256, 512, 512
NCORES = 8
BL = B // NCORES            # 128 batch items per core
BGRP = 4                    # batch items per h-block
RBLK = BGRP * N             # 400 rows per h-block
NBLK = BL // BGRP           # 32 h-blocks
NG = 3 * BL                 # 384 g rows (h0+hn | h0 | hn)
NF = 2 * BL                 # 256 f rows (cand-major)

H_DIMS = (XD, HID, HID, HID, HID, HD)
G_DIMS = (HD, HID, HID, HID, HID, HID, GD)
F_DIMS = (GD + 2 * HD, HID, HID, HID, HID, HID, 1)


def _declare_weights(nc, prefix, dims, last_bias):
    ws, bs = [], []
    for i, (di, do) in enumerate(zip(dims[:-1], dims[1:])):
        ws.append(nc.declare_dram_parameter(f"{prefix}W{i}", [di, do], MMDT, False))
        if i < len(dims) - 2 or last_bias:
            bs.append(nc.declare_dram_parameter(f"{prefix}b{i}", [do], F32, False))
        else:
            bs.append(None)
    return ws, bs


def _load_weights(nc, pool, ws, bs):
    """DMA weights into SBUF as [128, nk*do] lhsT chunk tiles, biases [128, nm]."""
    wts, bts = [], []
    for i, w in enumerate(ws):
        di, do = w.shape
        if di < 128:
            t = pool.tile([di, do], MMDT, tag=f"w{w.name}")
            nc.sync.dma_start(t[:], w[:])
        else:
            nk = di // 128
            t = pool.tile([128, nk * do], MMDT, tag=f"w{w.name}")
            nc.sync.dma_start(
                t[:].rearrange("p (k c) -> p k c", k=nk),
                w[:].rearrange("(k p) c -> p k c", p=128),
            )
        wts.append(t)
        b = bs[i]
        if b is None:
            bts.append(None)
            continue
        do = b.shape[0]
        if do >= 128:
            nm = do // 128
            bt = pool.tile([128, nm], F32, tag=f"b{b.name}")
            nc.sync.dma_start(bt[:], b[:].rearrange("(m p) -> p m", p=128))
        else:
            bt = pool.tile([do, 1], F32, tag=f"b{b.name}")
            nc.sync.dma_start(bt[:], b[:].rearrange("p -> p 1"))
        bts.append(bt)
    return wts, bts


def _mlp_layer(nc, ps_pool, act_pool, wt, bt, act_in, nrows, di, do,
               out_dtype=None, func=AF.Prelu, out_tile=None, tag=None):
    if out_dtype is None:
        out_dtype = MMDT
    """One feature-major linear(+bias) layer with fused activation.

    act_in: SBUF tile [128, nk*nrows] (k-chunk c at cols [c*nrows,(c+1)*nrows)).
    Returns SBUF tile [128, nm*nrows] in out_dtype.
    """
    nk = max(1, di // 128)
    nm = (do + 127) // 128
    mm = min(do, 128)
    if out_tile is None:
        out_tile = act_pool.tile([128, nm * nrows], out_dtype, tag=tag)
    psums = []
    for _m in range(nm):
        pt = ps_pool.tile([mm, nrows] if mm < 128 else [128, nrows], F32,
                          tag="ps", name=f"ps_{_m}")
        psums.append(pt)
    for k in range(nk):
        if di < 128:  # single short-K chunk (h layer 0: K=2)
            rhs = act_in[:]
        else:
            rhs = act_in[:, k * nrows:(k + 1) * nrows]
        for m in range(nm):
            lhsT = wt[:, k * do + m * mm: k * do + (m + 1) * mm]
            nc.tensor.matmul(psums[m][:], lhsT, rhs,
                             start=(k == 0), stop=(k == nk - 1))
    for m in range(nm):
        dst = out_tile[:, m * nrows:(m + 1) * nrows] if mm == 128 else out_tile[:]
        bias = bt[:, m:m + 1] if bt is not None else 0.0
        nc.scalar.activation(dst, psums[m][:], func, bias=bias, scale=1.0,
                             alpha=0.25)
    return out_tile


def build_program(loop_reps=None):
    nc = bass.Bass()
    data = nc.declare_dram_parameter("data", [BL, N, XD], MMDT, False)
    out = nc.declare_dram_parameter("out", [2, BL], F32, True)

    hW, hB = _declare_weights(nc, "h", H_DIMS, True)
    gW, gB = _declare_weights(nc, "g", G_DIMS, True)
    fW, fB = _declare_weights(nc, "f", F_DIMS, False)

    from contextlib import ExitStack
    with tile.TileContext(nc) as tc, ExitStack() as ctx:
        dump_sem = nc.alloc_semaphore("wsp_dump")
        wpool = ctx.enter_context(tc.tile_pool(name="weights", bufs=1))
        apool = ctx.enter_context(tc.tile_pool(name="acts", bufs=2))
        spool = ctx.enter_context(tc.tile_pool(name="stats", bufs=1))
        xpool = ctx.enter_context(tc.tile_pool(name="xt", bufs=3))
        hspool = ctx.enter_context(tc.tile_pool(name="hs", bufs=2))
        pspool = ctx.enter_context(tc.tile_pool(name="psum", bufs=8, space="PSUM"))

        hWt, hBt = _load_weights(nc, wpool, hW, hB)
        gWt, gBt = _load_weights(nc, wpool, gW, gB)
        fWt, fBt = _load_weights(nc, wpool, fW, fB)

        if loop_reps is not None:
            loop_cm = tc.For_i(0, loop_reps, 1)
            loop_cm.__enter__()

        # Accumulators across h blocks (feature-major [128, 2*BL]; HD=256 -> 2 chunks)
        nmh = HD // 128
        q_sum = spool.tile([128, nmh * BL], F32, tag="q_sum")
        h0_all = spool.tile([128, nmh * BL], F32, tag="h0")
        hn_all = spool.tile([128, nmh * BL], F32, tag="hn")

        data_fbn = data[:].rearrange("b n f -> f b n")

        for blk in range(NBLK):
            xt = xpool.tile([XD, RBLK], MMDT, tag="xt")
            nc.sync.dma_start(
                xt[:].rearrange("f (b n) -> f b n", b=BGRP),
                data_fbn[:, blk * BGRP:(blk + 1) * BGRP, :],
            )
            a = _mlp_layer(nc, pspool, apool, hWt[0], hBt[0], xt, RBLK,
                           XD, HID, tag="hact")
            for l in (1, 2, 3):
                a = _mlp_layer(nc, pspool, apool, hWt[l], hBt[l], a, RBLK,
                               HID, HID, tag="hact")
            hs = _mlp_layer(nc, pspool, hspool, hWt[4], hBt[4], a, RBLK,
                            HID, HD, out_dtype=F32, func=AF.Identity, tag="hs")
            # per-block reduction: sum over n, extract n=0 / n=1
            for m in range(nmh):
                col = m * BL + blk * BGRP
                nc.vector.tensor_reduce(
                    q_sum[:, col:col + BGRP],
                    hs[:, m * RBLK:(m + 1) * RBLK].rearrange(
                        "p (b n) -> p b n", b=BGRP),
                    mybir.AxisListType.X, ALU.add,
                )
                src = hs[:].rearrange("p (m b n) -> p m b n", m=nmh, b=BGRP)
                nc.vector.tensor_copy(h0_all[:, col:col + BGRP], src[:, m, :, 0])
                nc.vector.tensor_copy(hn_all[:, col:col + BGRP], src[:, m, :, 1])

        # Q = sum_{n>=2} = q_sum - h0 - hn   (keep f32)
        q = spool.tile([128, nmh * BL], F32, tag="q")
        nc.vector.tensor_tensor(q[:], q_sum[:], h0_all[:], ALU.subtract)
        nc.vector.tensor_tensor(q[:], q[:], hn_all[:], ALU.subtract)

        # g input [128, nmh * 384]: chunks (h0+hn | h0 | hn) per feature-half
        g_in = spool.tile([128, nmh * NG], MMDT, tag="g_in")
        for m in range(nmh):
            base = m * NG
            mh = slice(m * BL, (m + 1) * BL)
            nc.vector.tensor_tensor(g_in[:, base:base + BL],
                                    h0_all[:, mh], hn_all[:, mh], ALU.add)
            nc.vector.tensor_copy(g_in[:, base + BL:base + 2 * BL], h0_all[:, mh])
            nc.vector.tensor_copy(g_in[:, base + 2 * BL:base + 3 * BL], hn_all[:, mh])

        a = g_in
        dims = G_DIMS
        for l in range(6):
            last = l == 5
            a = _mlp_layer(nc, pspool, apool, gWt[l], gBt[l], a, NG,
                           dims[l], dims[l + 1],
                           out_dtype=F32 if last else MMDT,
                           func=AF.Identity if last else AF.Prelu,
                           tag="gact")

        g_out = a  # [128, 4*NG] f32

        # f input [128, 8 * 256] f32r: chunks 0-3 Gk, 4-5 Q, 6-7 hn
        f_in = spool.tile([128, 8 * NF], MMDT, tag="f_in")
        nmg = GD // 128
        for kc in range(nmg):
            gbase = kc * NG
            fbase = kc * NF
            nc.vector.tensor_copy(f_in[:, fbase:fbase + BL],
                                  g_out[:, gbase:gbase + BL])
            nc.vector.tensor_tensor(f_in[:, fbase + BL:fbase + 2 * BL],
                                    g_out[:, gbase + BL:gbase + 2 * BL],
                                    g_out[:, gbase + 2 * BL:gbase + 3 * BL],
                                    ALU.add)
        for m in range(nmh):
            fbase = (nmg + m) * NF
            mh = slice(m * BL, (m + 1) * BL)
            nc.vector.tensor_copy(f_in[:, fbase:fbase + BL], q[:, mh])
            nc.vector.tensor_copy(f_in[:, fbase + BL:fbase + 2 * BL], q[:, mh])
            fbase = (nmg + nmh + m) * NF
            nc.vector.tensor_copy(f_in[:, fbase:fbase + BL], hn_all[:, mh])
            nc.vector.tensor_copy(f_in[:, fbase + BL:fbase + 2 * BL], hn_all[:, mh])

        a = f_in
        dims = F_DIMS
        for l in range(5):
            a = _mlp_layer(nc, pspool, apool, fWt[l], fBt[l], a, NF,
                           dims[l], dims[l + 1], tag="fact")
        # final layer 512 -> 1, bias-free: logits psum [1, 256]
        lp = pspool.tile([1, NF], F32, tag="ps")
        for k in range(4):
            nc.tensor.matmul(lp[:], fWt[5][:, k:k + 1],
                             a[:, k * NF:(k + 1) * NF],
                             start=(k == 0), stop=(k == 3))

        # log_softmax over the 2 candidates (cols [0:BL) vs [BL:2BL))
        sm = spool.tile([1, 8 * BL], F32, tag="sm")
        l0 = sm[:, 0 * BL:1 * BL]; l1 = sm[:, 1 * BL:2 * BL]
        mx = sm[:, 2 * BL:3 * BL]; d0 = sm[:, 3 * BL:4 * BL]
        d1 = sm[:, 4 * BL:5 * BL]; e0 = sm[:, 5 * BL:6 * BL]
        e1 = sm[:, 6 * BL:7 * BL]; ls = sm[:, 7 * BL:8 * BL]
        nc.vector.tensor_copy(l0, lp[:, 0:BL])
        nc.vector.tensor_copy(l1, lp[:, BL:2 * BL])
        nc.vector.tensor_tensor(mx, l0, l1, ALU.max)
        nc.vector.tensor_tensor(d0, l0, mx, ALU.subtract)
        nc.vector.tensor_tensor(d1, l1, mx, ALU.subtract)
        nc.scalar.activation(e0, d0, AF.Exp)
        nc.scalar.activation(e1, d1, AF.Exp)
        nc.vector.tensor_tensor(e0, e0, e1, ALU.add)
        nc.scalar.activation(ls, e0, AF.Ln)
        ob = spool.tile([1, 2 * BL], F32, tag="ob")
        nc.vector.tensor_tensor(ob[:, 0:BL], d0, ls, ALU.subtract)
        nc.vector.tensor_tensor(ob[:, BL:2 * BL], d1, ls, ALU.subtract)
        nc.sync.dma_start(out[:].rearrange("a b -> (a b)").rearrange("(o c) -> o c", o=1), ob[:])

        if loop_reps is not None:
            loop_cm.__exit__(None, None, None)

    split_waits(nc, dump_sem)
    return nc


_nc_cache = {}


def _get_nc(loop_reps=None):
    if loop_reps not in _nc_cache:
        _nc_cache[loop_reps] = build_program(loop_reps)
    return _nc_cache[loop_reps]


def kernel(data, cs, n, h_params, g_params, f_params, loop_reps=None, **_ignored):
    nc = _get_nc(loop_reps)
    mmdt_np = mybir.dt.np(MMDT)
    data = np.ascontiguousarray(np.asarray(data, dtype=np.float32).astype(mmdt_np))

    base = {}
    for prefix, params in (("h", h_params), ("g", g_params), ("f", f_params)):
        ws, bs = params[0], params[1]
        for i, w in enumerate(ws):
            base[f"{prefix}W{i}"] = np.ascontiguousarray(
                np.asarray(w, np.float32).astype(mmdt_np))
        for i, b in enumerate(bs):
            if b is not None:
                base[f"{prefix}b{i}"] = np.ascontiguousarray(np.asarray(b, np.float32))

    import os
    in_maps = [dict(base, data=data[c * BL:(c + 1) * BL]) for c in range(NCORES)]
    trace = bool(int(os.environ.get("KERNEL_TRACE", "0")))
    res = run_bass_kernel_spmd(nc, in_maps, list(range(NCORES)), trace=trace)
    global _last_exec_ns, _last_results
    _last_exec_ns = res.exec_time_ns
    _last_results = res
    outp = np.empty((B, 2), dtype=np.float32)
    for c in range(NCORES):
        outp[c * BL:(c + 1) * BL, :] = res.results[c]["out"].T
    return outp
